# revision 17
# baseline (speedup 1.0000x reference)
"""Trainium2 Bass kernel for LongRangeTCN (4-layer dilated causal conv + BN + LIF + residual).

Sharding: data-parallel over batch B=32 -> 4 per core across 8 NeuronCores.

Design (per core):
  X   [C, BL, SX]  float32r residual state (conv halo zero pad at head); engine
                   writes round to f32r (~2.5e-4 rel) which is within noise budget.
  XH  [C, BL, SXH] fp32 conv output -> overwritten in place by the LIF A-trajectory.
  conv = 3 float32r matmuls/tile accumulated in PSUM (1 cycle/row = 4x over fp32);
  evac = ACT: XH = psum + bias (BN bias and the 0.5 LIF feed scale folded on host);
  scan = fused LIF step, ONE custom DVE op per step:
             A_j = select(A_{j-1} < 1, A_{j-1}, 0)*0.5 + xh_j
         chunk-parallel (64 chunks x 64 steps per batch) with a 16-step warmup
         (0.5 decay/step kills chunk-boundary state error below fp32 noise);
         real steps write A in place over xh in XH.
  res  = X = (XH >= 1) + X. Stream 0: fused DVE op per 512-tile (woven into
         stream 1's scan). Stream 1: spike S = Sigmoid(2^24*(XH-1)) on ACT
         (exact step at fp32 resolution) + X = I@X + I@S via f32r identity
         matmuls on PE, evacuated by ACT -- keeps the residual off the DVE.
Two independent batch streams software-pipeline: stream s+1's conv (PE/ACT/DMA)
runs under stream s's scan (DVE).
"""

import numpy as np

TAU, VTH, EPS, K = 2.0, 1.0, 1e-5, 3
DILATIONS = (1, 2, 4, 8)
B, C, T = 32, 128, 4096
NCORES = 8
BL = B // NCORES          # 4 batches per core
H = 16                    # scan warmup steps (0.5^H = 1.5e-5 kills carry)
PAD0 = 64                 # zero-pad columns at the head of each batch row in XH
LC = 64                   # scan chunk length
NC2 = T // LC             # 64 chunks per batch
PADX = 16                 # conv left halo (max (K-1)*d = 16)
SX = PADX + T             # 4112
SXH = PAD0 + T            # 4160 = 65*LC
OFF = PAD0 - H            # 48: step j of chunk c reads/writes col c*LC + OFF + j
NS = 2                    # batch streams per core
BS = BL // NS             # 2 batches per stream
TW = 512                  # conv tile width

_cache = {}


def _ensure_lif_op():
    """Register the fused LIF-step custom DVE op (idempotent)."""
    import concourse.dve_ops as dve_ops
    from concourse.dve_spec import Spec, Src0, Src1, C0, Zero, One, select, lower
    from concourse.dve_uop import DveOpSpec

    name = "LIF_STEP_ANT"
    for o in dve_ops.OPS:
        if o.name == name:
            return o
    body = select(Src1 < One, Src1, Zero) * C0 + Src0
    spec = Spec(
        body=body,
        reference=lambda in0, in1, s0, s1, imm2: (
            np.where(in1 < 1.0, in1, 0.0).astype(np.float32) * np.float32(s0) + in0
        ),
    )
    row = dve_ops._CUSTOM_DVE_ROW_BASE + len(dve_ops.OPS)
    shas = {}
    for ver in ("v3", "v4"):
        u = lower(spec, ver=ver)
        s = DveOpSpec(name=name, opcode=row, uops=u, rd1_en=True)
        shas[ver] = s.sha(ver)
    op = dve_ops.DveOp(name, spec, subdim=False, uops_sha=shas)
    dve_ops.OPS.append(op)
    dve_ops.CUSTOM_DVE_SPECS[name] = spec
    dve_ops._SUB_OPCODE_FOR_NAME[name] = row
    return op


def _build():
    import concourse.bass as bass
    import concourse.bacc as bacc
    import concourse.tile as tile
    import concourse.mybir as mybir

    LIF = _ensure_lif_op()

    dt = mybir.dt.float32
    dtr = mybir.dt.float32r
    Alu = mybir.AluOpType
    Act = mybir.ActivationFunctionType

    nc = bacc.Bacc("TRN2", target_bir_lowering=False, debug=False)
    x_d = nc.dram_tensor("x", [BL, C, SX], dtr, kind="ExternalInput")  # host pre-pads halo
    wt_d = nc.dram_tensor("wt", [C, 4, K, C], dtr, kind="ExternalInput")
    b_d = nc.dram_tensor("bias", [C, 4], dt, kind="ExternalInput")
    id_d = nc.dram_tensor("idm", [C, C], dtr, kind="ExternalInput")
    o_d = nc.dram_tensor("out", [BL, C, T], dtr, kind="ExternalOutput")

    with tile.TileContext(nc) as tc:
        with (
            tc.tile_pool(name="big", bufs=1) as big,
            tc.tile_pool(name="small", bufs=1) as small,
            tc.tile_pool(name="psum", bufs=4, space="PSUM") as pp,
        ):
            X = big.tile([C, BL, SX], dtr, tag="X")
            XH = big.tile([C, BL, SXH], dt, tag="XH")
            WT = small.tile([C, 4, K, C], dtr, tag="WT")
            BIAS = small.tile([C, 4], dt, tag="BIAS")
            ZERO = small.tile([C, BL, NC2], dt, tag="ZERO")
            S = big.tile([C, BS, T], dtr, tag="S")
            IDM = small.tile([C, C], dtr, tag="IDM")
            NBIG = small.tile([C, 1], dt, tag="NBIG")
            SCR = [
                [
                    small.tile([C, BS, NC2], dt, name=f"SCR{s}{i}", tag=f"SCR{s}{i}")
                    for i in range(2)
                ]
                for s in range(NS)
            ]

            BIG = float(2.0 ** 24)
            # issue order matters: each DMA costs ~4us of queue-issue time,
            # so the conv-critical transfers (WT, X b0/b1) go first; BIAS is
            # needed only by the first evac, IDM only by the first res_act.
            nc.sync.dma_start(WT[:], wt_d[:])
            nc.sync.dma_start(X[:, 0, :], x_d[0])
            nc.sync.dma_start(X[:, 1, :], x_d[1])
            nc.sync.dma_start(BIAS[:], b_d[:])
            nc.sync.dma_start(X[:, 2, :], x_d[2])
            nc.sync.dma_start(X[:, 3, :], x_d[3])
            nc.sync.dma_start(IDM[:], id_d[:])
            nc.vector.memset(NBIG[:], -BIG)
            nc.vector.memset(XH[:, :, 0:PAD0], 0.0)
            nc.vector.memset(ZERO[:], 0.0)

            XH4 = XH[:].rearrange("p a (c l) -> p a c l", l=LC)
            WTf = WT[:].rearrange("p a k c -> p (a k c)")
            psw = pp.tile([C, TW], dt, tag="psw")
            for i in range(11):
                nc.tensor.matmul(psw[:], WT[:, 0, 0, :], WTf[:, 0:TW],
                                 start=True, stop=True)

            def conv(li, b):
                for t0 in range(0, T, TW):
                    ps = pp.tile([C, TW], dt, tag="ps")
                    for k in range(K):
                        sh = (K - 1 - k) * DILATIONS[li]
                        nc.tensor.matmul(
                            ps[:],
                            WT[:, li, k, :],
                            X[:, b, PADX + t0 - sh : PADX + t0 - sh + TW],
                            start=(k == 0),
                            stop=(k == K - 1),
                        )
                    nc.scalar.activation(
                        XH[:, b, PAD0 + t0 : PAD0 + t0 + TW], ps[:],
                        Act.Identity, bias=BIAS[:, li : li + 1], scale=1.0,
                    )

            def scan_ops(s):
                """Yield the H+LC chained LIF-step ops for stream s."""
                b0 = s * BS
                for j in range(H + LC):
                    jj = OFF + j
                    if jj < LC:
                        col = XH4[:, b0 : b0 + BS, 0:NC2, jj]
                    else:
                        col = XH4[:, b0 : b0 + BS, 1 : NC2 + 1, jj - LC]
                    if j == 0:
                        prev = ZERO[:, b0 : b0 + BS, :]
                    elif j <= H:
                        prev = SCR[s][(j - 1) % 2][:]
                    else:
                        jp = OFF + j - 1
                        if jp < LC:
                            prev = XH4[:, b0 : b0 + BS, 0:NC2, jp]
                        else:
                            prev = XH4[:, b0 : b0 + BS, 1 : NC2 + 1, jp - LC]
                    dst = SCR[s][j % 2][:] if j < H else col
                    yield lambda col=col, prev=prev, dst=dst: nc.vector._custom_dve(
                        LIF, out=dst, in0=col, in1=prev, s0=0.5
                    )

            def res_ops_batch(b):
                for t0 in range(0, T, TW):
                    yield lambda b=b, t0=t0: nc.vector.scalar_tensor_tensor(
                        X[:, b, PADX + t0 : PADX + t0 + TW],
                        XH[:, b, PAD0 + t0 : PAD0 + t0 + TW], float(VTH),
                        X[:, b, PADX + t0 : PADX + t0 + TW],
                        op0=Alu.is_ge, op1=Alu.add,
                    )

            def res_act(b):
                """Batch-b residual via ACT sigmoid spike + PE identity add:
                S = step(XH - 1); X = I@X + I@S (exact; evac rounds to f32r
                exactly like the DVE path)."""
                sb_ = b % BS
                nc.scalar.activation(
                    S[:, sb_, :], XH[:, b, PAD0 : PAD0 + T],
                    Act.Sigmoid, bias=NBIG[:], scale=BIG,
                )
                for t0 in range(0, T, TW):
                    ps = pp.tile([C, TW], dt, tag="ps")
                    nc.tensor.matmul(
                        ps[:], IDM[:], X[:, b, PADX + t0 : PADX + t0 + TW],
                        start=True, stop=False,
                    )
                    nc.tensor.matmul(
                        ps[:], IDM[:], S[:, sb_, t0 : t0 + TW],
                        start=False, stop=True,
                    )
                    nc.scalar.activation(
                        X[:, b, PADX + t0 : PADX + t0 + TW], ps[:],
                        Act.Identity, bias=0.0, scale=1.0,
                    )

            def res_ops(s):
                """Yield X = (XH >= 1) + X per 512-tile for stream s."""
                for b in range(s * BS, (s + 1) * BS):
                    for t0 in range(0, T, TW):
                        yield lambda b=b, t0=t0: nc.vector.scalar_tensor_tensor(
                            X[:, b, PADX + t0 : PADX + t0 + TW],
                            XH[:, b, PAD0 + t0 : PAD0 + t0 + TW], float(VTH),
                            X[:, b, PADX + t0 : PADX + t0 + TW],
                            op0=Alu.is_ge, op1=Alu.add,
                        )

            def weave(scan_iter, res_iter, every=5):
                """Interleave res tiles into the head of the chained scan
                stream so the other stream's next conv+evac completes well
                before this scan ends."""
                n = 0
                for op in scan_iter:
                    op()
                    n += 1
                    if n % every == 0:
                        r = next(res_iter, None)
                        if r is not None:
                            r()
                for r in res_iter:
                    r()

            # software pipeline over (layer, stream) slots; streams are fully
            # independent end-to-end, so slot k's scan hides slot k-1's res
            # (woven) and slot k+1's conv (PE/ACT).
            slots = [(li, s) for li in range(len(DILATIONS)) for s in range(NS)]
            for b in range(0, BS):
                conv(0, b)
            for k, (li, s) in enumerate(slots):
                if s == 0:
                    # weave batch-2's previous-layer DVE res into this scan
                    prev_b2 = res_ops_batch(BS) if li >= 1 else iter(())
                    weave(scan_ops(0), prev_b2)
                    for b in range(BS, BL):
                        conv(li, b)           # same layer, stream 1
                else:
                    # stream 0's DVE res weaves into stream 1's scan
                    weave(scan_ops(1), res_ops(0))
                    if li == len(DILATIONS) - 1:
                        for b in range(0, BS):
                            nc.sync.dma_start(o_d[b], X[:, b, PADX:SX])
                        for b in range(BS, BL):
                            for r in res_ops_batch(b):  # final: all-DVE
                                r()
                            # last batch's output goes out via the idle ACT
                            # queue so its issue overlaps b2's transfer
                            q = nc.sync if b == BS else nc.scalar
                            q.dma_start(o_d[b], X[:, b, PADX:SX])
                    else:
                        for b in range(0, BS):
                            conv(li + 1, b)   # next layer, stream 0
                        res_act(BS + 1)       # batch 3 residual on ACT+PE

    nc.compile()
    return nc


def kernel(x, w, gamma, beta, mean, var, **_):
    from concourse.bass_utils import run_bass_kernel_spmd

    x = np.pad(np.asarray(x, np.float32), ((0, 0), (0, 0), (PADX, 0)))
    inv = (gamma / np.sqrt(var + EPS)).astype(np.float32)          # [4, C]
    # wt[ci, l, k, co] = 0.5 * w[l, co, ci, k] * inv[l, co]
    wt = (0.5 * w * inv[:, :, None, None]).astype(np.float32)      # [4, Co, Ci, K]
    wt = np.ascontiguousarray(wt.transpose(2, 0, 3, 1))            # [Ci, 4, K, Co]
    bias = (0.5 * (beta - mean * inv)).astype(np.float32).T        # [C, 4]
    bias = np.ascontiguousarray(bias)

    if "nc" not in _cache:
        _cache["nc"] = _build()
    nc = _cache["nc"]

    idm = np.eye(C, dtype=np.float32)
    in_maps = [
        {"x": np.ascontiguousarray(x[i * BL : (i + 1) * BL]), "wt": wt,
         "bias": bias, "idm": idm}
        for i in range(NCORES)
    ]
    res = run_bass_kernel_spmd(nc, in_maps, list(range(NCORES)))
    return np.concatenate([res.results[i]["out"] for i in range(NCORES)], axis=0)


# revision 18
# speedup vs baseline: 1.0099x; 1.0099x over previous
"""Trainium2 Bass kernel for LongRangeTCN (4-layer dilated causal conv + BN + LIF + residual).

Sharding: data-parallel over batch B=32 -> 4 per core across 8 NeuronCores.

Design (per core):
  X   [C, BL, SX]  float32r residual state (conv halo zero pad at head); engine
                   writes round to f32r (~2.5e-4 rel) which is within noise budget.
  XH  [C, BL, SXH] fp32 conv output -> overwritten in place by the LIF A-trajectory.
  conv = 3 float32r matmuls/tile accumulated in PSUM (1 cycle/row = 4x over fp32);
  evac = ACT: XH = psum + bias (BN bias and the 0.5 LIF feed scale folded on host);
  scan = fused LIF step, ONE custom DVE op per step:
             A_j = select(A_{j-1} < 1, A_{j-1}, 0)*0.5 + xh_j
         chunk-parallel (64 chunks x 64 steps per batch) with a 16-step warmup
         (0.5 decay/step kills chunk-boundary state error below fp32 noise);
         real steps write A in place over xh in XH.
  res  = X = (XH >= 1) + X. Stream 0: fused DVE op per 512-tile (woven into
         stream 1's scan). Stream 1: spike S = Sigmoid(2^24*(XH-1)) on ACT
         (exact step at fp32 resolution) + X = I@X + I@S via f32r identity
         matmuls on PE, evacuated by ACT -- keeps the residual off the DVE.
Two independent batch streams software-pipeline: stream s+1's conv (PE/ACT/DMA)
runs under stream s's scan (DVE).
"""

import numpy as np

TAU, VTH, EPS, K = 2.0, 1.0, 1e-5, 3
DILATIONS = (1, 2, 4, 8)
B, C, T = 32, 128, 4096
NCORES = 8
BL = B // NCORES          # 4 batches per core
H = 16                    # scan warmup steps (0.5^H = 1.5e-5 kills carry)
PAD0 = 64                 # zero-pad columns at the head of each batch row in XH
LC = 64                   # scan chunk length
NC2 = T // LC             # 64 chunks per batch
PADX = 16                 # conv left halo (max (K-1)*d = 16)
SX = PADX + T             # 4112
SXH = PAD0 + T            # 4160 = 65*LC
OFF = PAD0 - H            # 48: step j of chunk c reads/writes col c*LC + OFF + j
NS = 2                    # batch streams per core
BS = BL // NS             # 2 batches per stream
TW = 512                  # conv tile width

_cache = {}


def _ensure_lif_op():
    """Register the fused LIF-step custom DVE op (idempotent)."""
    import concourse.dve_ops as dve_ops
    from concourse.dve_spec import Spec, Src0, Src1, C0, Zero, One, select, lower
    from concourse.dve_uop import DveOpSpec

    name = "LIF_STEP_ANT"
    for o in dve_ops.OPS:
        if o.name == name:
            return o
    body = select(Src1 < One, Src1, Zero) * C0 + Src0
    spec = Spec(
        body=body,
        reference=lambda in0, in1, s0, s1, imm2: (
            np.where(in1 < 1.0, in1, 0.0).astype(np.float32) * np.float32(s0) + in0
        ),
    )
    row = dve_ops._CUSTOM_DVE_ROW_BASE + len(dve_ops.OPS)
    shas = {}
    for ver in ("v3", "v4"):
        u = lower(spec, ver=ver)
        s = DveOpSpec(name=name, opcode=row, uops=u, rd1_en=True)
        shas[ver] = s.sha(ver)
    op = dve_ops.DveOp(name, spec, subdim=False, uops_sha=shas)
    dve_ops.OPS.append(op)
    dve_ops.CUSTOM_DVE_SPECS[name] = spec
    dve_ops._SUB_OPCODE_FOR_NAME[name] = row
    return op


def _build():
    import concourse.bass as bass
    import concourse.bacc as bacc
    import concourse.tile as tile
    import concourse.mybir as mybir

    LIF = _ensure_lif_op()

    dt = mybir.dt.float32
    dtr = mybir.dt.float32r
    Alu = mybir.AluOpType
    Act = mybir.ActivationFunctionType

    nc = bacc.Bacc("TRN2", target_bir_lowering=False, debug=False)
    x_d = nc.dram_tensor("x", [BL, C, SX], dtr, kind="ExternalInput")  # host pre-pads halo
    wt_d = nc.dram_tensor("wt", [C, 4, K, C], dtr, kind="ExternalInput")
    b_d = nc.dram_tensor("bias", [C, 4], dt, kind="ExternalInput")
    id_d = nc.dram_tensor("idm", [C, C], dtr, kind="ExternalInput")
    o_d = nc.dram_tensor("out", [BL, C, T], dtr, kind="ExternalOutput")

    with tile.TileContext(nc) as tc:
        with (
            tc.tile_pool(name="big", bufs=1) as big,
            tc.tile_pool(name="small", bufs=1) as small,
            tc.tile_pool(name="psum", bufs=4, space="PSUM") as pp,
        ):
            X = big.tile([C, BL, SX], dtr, tag="X")
            XH = big.tile([C, BL, SXH], dt, tag="XH")
            WT = small.tile([C, 4, K, C], dtr, tag="WT")
            BIAS = small.tile([C, 4], dt, tag="BIAS")
            ZERO = small.tile([C, BL, NC2], dt, tag="ZERO")
            S = big.tile([C, BS, T], dtr, tag="S")
            IDM = small.tile([C, C], dtr, tag="IDM")
            NBIG = small.tile([C, 1], dt, tag="NBIG")
            SCR = [
                [
                    small.tile([C, BS, NC2], dt, name=f"SCR{s}{i}", tag=f"SCR{s}{i}")
                    for i in range(2)
                ]
                for s in range(NS)
            ]

            BIG = float(2.0 ** 24)
            nc.sync.dma_start(WT[:], wt_d[:])
            nc.sync.dma_start(BIAS[:], b_d[:])
            nc.sync.dma_start(IDM[:], id_d[:])
            nc.vector.memset(NBIG[:], -BIG)
            nc.vector.memset(XH[:, :, 0:PAD0], 0.0)
            nc.vector.memset(ZERO[:], 0.0)
            for b in range(BL):
                nc.sync.dma_start(X[:, b, :], x_d[b])

            XH4 = XH[:].rearrange("p a (c l) -> p a c l", l=LC)
            WTf = WT[:].rearrange("p a k c -> p (a k c)")
            psw = pp.tile([C, TW], dt, tag="psw")
            for i in range(11):
                nc.tensor.matmul(psw[:], WT[:, 0, 0, :], WTf[:, 0:TW],
                                 start=True, stop=True)

            def conv(li, b):
                for t0 in range(0, T, TW):
                    ps = pp.tile([C, TW], dt, tag="ps")
                    for k in range(K):
                        sh = (K - 1 - k) * DILATIONS[li]
                        nc.tensor.matmul(
                            ps[:],
                            WT[:, li, k, :],
                            X[:, b, PADX + t0 - sh : PADX + t0 - sh + TW],
                            start=(k == 0),
                            stop=(k == K - 1),
                        )
                    nc.scalar.activation(
                        XH[:, b, PAD0 + t0 : PAD0 + t0 + TW], ps[:],
                        Act.Identity, bias=BIAS[:, li : li + 1], scale=1.0,
                    )

            def scan_ops(s):
                """Yield the H+LC chained LIF-step ops for stream s."""
                b0 = s * BS
                for j in range(H + LC):
                    jj = OFF + j
                    if jj < LC:
                        col = XH4[:, b0 : b0 + BS, 0:NC2, jj]
                    else:
                        col = XH4[:, b0 : b0 + BS, 1 : NC2 + 1, jj - LC]
                    if j == 0:
                        prev = ZERO[:, b0 : b0 + BS, :]
                    elif j <= H:
                        prev = SCR[s][(j - 1) % 2][:]
                    else:
                        jp = OFF + j - 1
                        if jp < LC:
                            prev = XH4[:, b0 : b0 + BS, 0:NC2, jp]
                        else:
                            prev = XH4[:, b0 : b0 + BS, 1 : NC2 + 1, jp - LC]
                    dst = SCR[s][j % 2][:] if j < H else col
                    yield lambda col=col, prev=prev, dst=dst: nc.vector._custom_dve(
                        LIF, out=dst, in0=col, in1=prev, s0=0.5
                    )

            def res_ops_batch(b):
                for t0 in range(0, T, TW):
                    yield lambda b=b, t0=t0: nc.vector.scalar_tensor_tensor(
                        X[:, b, PADX + t0 : PADX + t0 + TW],
                        XH[:, b, PAD0 + t0 : PAD0 + t0 + TW], float(VTH),
                        X[:, b, PADX + t0 : PADX + t0 + TW],
                        op0=Alu.is_ge, op1=Alu.add,
                    )

            def res_act(b):
                """Batch-b residual via ACT sigmoid spike + PE identity add:
                S = step(XH - 1); X = I@X + I@S (exact; evac rounds to f32r
                exactly like the DVE path)."""
                sb_ = b % BS
                nc.scalar.activation(
                    S[:, sb_, :], XH[:, b, PAD0 : PAD0 + T],
                    Act.Sigmoid, bias=NBIG[:], scale=BIG,
                )
                for t0 in range(0, T, TW):
                    ps = pp.tile([C, TW], dt, tag="ps")
                    nc.tensor.matmul(
                        ps[:], IDM[:], X[:, b, PADX + t0 : PADX + t0 + TW],
                        start=True, stop=False,
                    )
                    nc.tensor.matmul(
                        ps[:], IDM[:], S[:, sb_, t0 : t0 + TW],
                        start=False, stop=True,
                    )
                    nc.scalar.activation(
                        X[:, b, PADX + t0 : PADX + t0 + TW], ps[:],
                        Act.Identity, bias=0.0, scale=1.0,
                    )

            def res_ops(s):
                """Yield X = (XH >= 1) + X per 512-tile for stream s."""
                for b in range(s * BS, (s + 1) * BS):
                    for t0 in range(0, T, TW):
                        yield lambda b=b, t0=t0: nc.vector.scalar_tensor_tensor(
                            X[:, b, PADX + t0 : PADX + t0 + TW],
                            XH[:, b, PAD0 + t0 : PAD0 + t0 + TW], float(VTH),
                            X[:, b, PADX + t0 : PADX + t0 + TW],
                            op0=Alu.is_ge, op1=Alu.add,
                        )

            def weave(scan_iter, res_iter, every=5):
                """Interleave res tiles into the head of the chained scan
                stream so the other stream's next conv+evac completes well
                before this scan ends."""
                n = 0
                for op in scan_iter:
                    op()
                    n += 1
                    if n % every == 0:
                        r = next(res_iter, None)
                        if r is not None:
                            r()
                for r in res_iter:
                    r()

            # software pipeline over (layer, stream) slots; streams are fully
            # independent end-to-end, so slot k's scan hides slot k-1's res
            # (woven) and slot k+1's conv (PE/ACT).
            slots = [(li, s) for li in range(len(DILATIONS)) for s in range(NS)]
            for b in range(0, BS):
                conv(0, b)
            for k, (li, s) in enumerate(slots):
                if s == 0:
                    # weave batch-2's previous-layer DVE res into this scan
                    prev_b2 = res_ops_batch(BS) if li >= 1 else iter(())
                    weave(scan_ops(0), prev_b2)
                    for b in range(BS, BL):
                        conv(li, b)           # same layer, stream 1
                else:
                    # stream 0's DVE res weaves into stream 1's scan
                    weave(scan_ops(1), res_ops(0))
                    if li == len(DILATIONS) - 1:
                        for b in range(0, BS):
                            nc.sync.dma_start(o_d[b], X[:, b, PADX:SX])
                        for b in range(BS, BL):
                            for r in res_ops_batch(b):  # final: all-DVE
                                r()
                            nc.sync.dma_start(o_d[b], X[:, b, PADX:SX])
                    else:
                        for b in range(0, BS):
                            conv(li + 1, b)   # next layer, stream 0
                        res_act(BS + 1)       # batch 3 residual on ACT+PE

    nc.compile()
    return nc


def kernel(x, w, gamma, beta, mean, var, **_):
    from concourse.bass_utils import run_bass_kernel_spmd

    x = np.pad(np.asarray(x, np.float32), ((0, 0), (0, 0), (PADX, 0)))
    inv = (gamma / np.sqrt(var + EPS)).astype(np.float32)          # [4, C]
    # wt[ci, l, k, co] = 0.5 * w[l, co, ci, k] * inv[l, co]
    wt = (0.5 * w * inv[:, :, None, None]).astype(np.float32)      # [4, Co, Ci, K]
    wt = np.ascontiguousarray(wt.transpose(2, 0, 3, 1))            # [Ci, 4, K, Co]
    bias = (0.5 * (beta - mean * inv)).astype(np.float32).T        # [C, 4]
    bias = np.ascontiguousarray(bias)

    if "nc" not in _cache:
        _cache["nc"] = _build()
    nc = _cache["nc"]

    idm = np.eye(C, dtype=np.float32)
    in_maps = [
        {"x": np.ascontiguousarray(x[i * BL : (i + 1) * BL]), "wt": wt,
         "bias": bias, "idm": idm}
        for i in range(NCORES)
    ]
    res = run_bass_kernel_spmd(nc, in_maps, list(range(NCORES)))
    return np.concatenate([res.results[i]["out"] for i in range(NCORES)], axis=0)


# revision 19
# speedup vs baseline: 1.0393x; 1.0291x over previous
"""Trainium2 Bass kernel for LongRangeTCN (4-layer dilated causal conv + BN + LIF + residual).

Sharding: data-parallel over batch B=32 -> 4 per core across 8 NeuronCores.

Design (per core):
  X   [C, BL, SX]  float32r residual state (conv halo zero pad at head); engine
                   writes round to f32r (~2.5e-4 rel) which is within noise budget.
  XH  [C, BL, SXH] fp32 conv output -> overwritten in place by the LIF A-trajectory.
  conv = 3 float32r matmuls/tile accumulated in PSUM (1 cycle/row = 4x over fp32);
  evac = ACT: XH = psum + bias (BN bias and the 0.5 LIF feed scale folded on host);
  scan = fused LIF step, ONE custom DVE op per step:
             A_j = select(A_{j-1} < 1, A_{j-1}, 0)*0.5 + xh_j
         chunk-parallel (64 chunks x 64 steps per batch) with a 16-step warmup
         (0.5 decay/step kills chunk-boundary state error below fp32 noise);
         real steps write A in place over xh in XH.
  res  = X = (XH >= 1) + X. Stream 0: fused DVE op per 512-tile (woven into
         stream 1's scan). Stream 1: spike S = Sigmoid(2^24*(XH-1)) on ACT
         (exact step at fp32 resolution) + X = I@X + I@S via f32r identity
         matmuls on PE, evacuated by ACT -- keeps the residual off the DVE.
Two independent batch streams software-pipeline: stream s+1's conv (PE/ACT/DMA)
runs under stream s's scan (DVE).
"""

import numpy as np

TAU, VTH, EPS, K = 2.0, 1.0, 1e-5, 3
DILATIONS = (1, 2, 4, 8)
B, C, T = 32, 128, 4096
NCORES = 8
BL = B // NCORES          # 4 batches per core
H = 12                    # scan warmup steps (0.5^H = 2.4e-4 carry residual)
PAD0 = 64                 # zero-pad columns at the head of each batch row in XH
LC = 64                   # scan chunk length
NC2 = T // LC             # 64 chunks per batch
PADX = 16                 # conv left halo (max (K-1)*d = 16)
SX = PADX + T             # 4112
SXH = PAD0 + T            # 4160 = 65*LC
OFF = PAD0 - H            # 48: step j of chunk c reads/writes col c*LC + OFF + j
NS = 2                    # batch streams per core
BS = BL // NS             # 2 batches per stream
TW = 512                  # conv tile width

_cache = {}


def _ensure_lif_op():
    """Register the fused LIF-step custom DVE op (idempotent)."""
    import concourse.dve_ops as dve_ops
    from concourse.dve_spec import Spec, Src0, Src1, C0, Zero, One, select, lower
    from concourse.dve_uop import DveOpSpec

    name = "LIF_STEP_ANT"
    for o in dve_ops.OPS:
        if o.name == name:
            return o
    body = select(Src1 < One, Src1, Zero) * C0 + Src0
    spec = Spec(
        body=body,
        reference=lambda in0, in1, s0, s1, imm2: (
            np.where(in1 < 1.0, in1, 0.0).astype(np.float32) * np.float32(s0) + in0
        ),
    )
    row = dve_ops._CUSTOM_DVE_ROW_BASE + len(dve_ops.OPS)
    shas = {}
    for ver in ("v3", "v4"):
        u = lower(spec, ver=ver)
        s = DveOpSpec(name=name, opcode=row, uops=u, rd1_en=True)
        shas[ver] = s.sha(ver)
    op = dve_ops.DveOp(name, spec, subdim=False, uops_sha=shas)
    dve_ops.OPS.append(op)
    dve_ops.CUSTOM_DVE_SPECS[name] = spec
    dve_ops._SUB_OPCODE_FOR_NAME[name] = row
    return op


def _build():
    import concourse.bass as bass
    import concourse.bacc as bacc
    import concourse.tile as tile
    import concourse.mybir as mybir

    LIF = _ensure_lif_op()

    dt = mybir.dt.float32
    dtr = mybir.dt.float32r
    Alu = mybir.AluOpType
    Act = mybir.ActivationFunctionType

    nc = bacc.Bacc("TRN2", target_bir_lowering=False, debug=False)
    x_d = nc.dram_tensor("x", [BL, C, SX], dtr, kind="ExternalInput")  # host pre-pads halo
    wt_d = nc.dram_tensor("wt", [C, 4, K, C], dtr, kind="ExternalInput")
    b_d = nc.dram_tensor("bias", [C, 4], dt, kind="ExternalInput")
    id_d = nc.dram_tensor("idm", [C, C], dtr, kind="ExternalInput")
    o_d = nc.dram_tensor("out", [BL, C, T], dtr, kind="ExternalOutput")

    with tile.TileContext(nc) as tc:
        with (
            tc.tile_pool(name="big", bufs=1) as big,
            tc.tile_pool(name="small", bufs=1) as small,
            tc.tile_pool(name="psum", bufs=4, space="PSUM") as pp,
        ):
            X = big.tile([C, BL, SX], dtr, tag="X")
            XH = big.tile([C, BL, SXH], dt, tag="XH")
            WT = small.tile([C, 4, K, C], dtr, tag="WT")
            BIAS = small.tile([C, 4], dt, tag="BIAS")
            ZERO = small.tile([C, BL, NC2], dt, tag="ZERO")
            S = big.tile([C, BS, T], dtr, tag="S")
            IDM = small.tile([C, C], dtr, tag="IDM")
            NBIG = small.tile([C, 1], dt, tag="NBIG")
            SCR = [
                [
                    small.tile([C, BS, NC2], dt, name=f"SCR{s}{i}", tag=f"SCR{s}{i}")
                    for i in range(2)
                ]
                for s in range(NS)
            ]

            BIG = float(2.0 ** 24)
            nc.sync.dma_start(WT[:], wt_d[:])
            nc.sync.dma_start(BIAS[:], b_d[:])
            nc.sync.dma_start(IDM[:], id_d[:])
            nc.vector.memset(NBIG[:], -BIG)
            nc.vector.memset(XH[:, :, 0:PAD0], 0.0)
            nc.vector.memset(ZERO[:], 0.0)
            for b in range(BL):
                nc.sync.dma_start(X[:, b, :], x_d[b])

            XH4 = XH[:].rearrange("p a (c l) -> p a c l", l=LC)
            WTf = WT[:].rearrange("p a k c -> p (a k c)")
            psw = pp.tile([C, TW], dt, tag="psw")
            for i in range(11):
                nc.tensor.matmul(psw[:], WT[:, 0, 0, :], WTf[:, 0:TW],
                                 start=True, stop=True)

            def conv(li, b):
                for t0 in range(0, T, TW):
                    ps = pp.tile([C, TW], dt, tag="ps")
                    for k in range(K):
                        sh = (K - 1 - k) * DILATIONS[li]
                        nc.tensor.matmul(
                            ps[:],
                            WT[:, li, k, :],
                            X[:, b, PADX + t0 - sh : PADX + t0 - sh + TW],
                            start=(k == 0),
                            stop=(k == K - 1),
                        )
                    nc.scalar.activation(
                        XH[:, b, PAD0 + t0 : PAD0 + t0 + TW], ps[:],
                        Act.Identity, bias=BIAS[:, li : li + 1], scale=1.0,
                    )

            def scan_ops(s):
                """Yield the H+LC chained LIF-step ops for stream s."""
                b0 = s * BS
                for j in range(H + LC):
                    jj = OFF + j
                    if jj < LC:
                        col = XH4[:, b0 : b0 + BS, 0:NC2, jj]
                    else:
                        col = XH4[:, b0 : b0 + BS, 1 : NC2 + 1, jj - LC]
                    if j == 0:
                        prev = ZERO[:, b0 : b0 + BS, :]
                    elif j <= H:
                        prev = SCR[s][(j - 1) % 2][:]
                    else:
                        jp = OFF + j - 1
                        if jp < LC:
                            prev = XH4[:, b0 : b0 + BS, 0:NC2, jp]
                        else:
                            prev = XH4[:, b0 : b0 + BS, 1 : NC2 + 1, jp - LC]
                    dst = SCR[s][j % 2][:] if j < H else col
                    yield lambda col=col, prev=prev, dst=dst: nc.vector._custom_dve(
                        LIF, out=dst, in0=col, in1=prev, s0=0.5
                    )

            def res_ops_batch(b):
                for t0 in range(0, T, TW):
                    yield lambda b=b, t0=t0: nc.vector.scalar_tensor_tensor(
                        X[:, b, PADX + t0 : PADX + t0 + TW],
                        XH[:, b, PAD0 + t0 : PAD0 + t0 + TW], float(VTH),
                        X[:, b, PADX + t0 : PADX + t0 + TW],
                        op0=Alu.is_ge, op1=Alu.add,
                    )

            def res_act(b):
                """Batch-b residual via ACT sigmoid spike + PE identity add:
                S = step(XH - 1); X = I@X + I@S (exact; evac rounds to f32r
                exactly like the DVE path)."""
                sb_ = b % BS
                nc.scalar.activation(
                    S[:, sb_, :], XH[:, b, PAD0 : PAD0 + T],
                    Act.Sigmoid, bias=NBIG[:], scale=BIG,
                )
                for t0 in range(0, T, TW):
                    ps = pp.tile([C, TW], dt, tag="ps")
                    nc.tensor.matmul(
                        ps[:], IDM[:], X[:, b, PADX + t0 : PADX + t0 + TW],
                        start=True, stop=False,
                    )
                    nc.tensor.matmul(
                        ps[:], IDM[:], S[:, sb_, t0 : t0 + TW],
                        start=False, stop=True,
                    )
                    nc.scalar.activation(
                        X[:, b, PADX + t0 : PADX + t0 + TW], ps[:],
                        Act.Identity, bias=0.0, scale=1.0,
                    )

            def res_ops(s):
                """Yield X = (XH >= 1) + X per 512-tile for stream s."""
                for b in range(s * BS, (s + 1) * BS):
                    for t0 in range(0, T, TW):
                        yield lambda b=b, t0=t0: nc.vector.scalar_tensor_tensor(
                            X[:, b, PADX + t0 : PADX + t0 + TW],
                            XH[:, b, PAD0 + t0 : PAD0 + t0 + TW], float(VTH),
                            X[:, b, PADX + t0 : PADX + t0 + TW],
                            op0=Alu.is_ge, op1=Alu.add,
                        )

            def weave(scan_iter, res_iter, every=5):
                """Interleave res tiles into the head of the chained scan
                stream so the other stream's next conv+evac completes well
                before this scan ends."""
                n = 0
                for op in scan_iter:
                    op()
                    n += 1
                    if n % every == 0:
                        r = next(res_iter, None)
                        if r is not None:
                            r()
                for r in res_iter:
                    r()

            # software pipeline over (layer, stream) slots; streams are fully
            # independent end-to-end, so slot k's scan hides slot k-1's res
            # (woven) and slot k+1's conv (PE/ACT).
            slots = [(li, s) for li in range(len(DILATIONS)) for s in range(NS)]
            for b in range(0, BS):
                conv(0, b)
            for k, (li, s) in enumerate(slots):
                if s == 0:
                    # weave batch-2's previous-layer DVE res into this scan
                    prev_b2 = res_ops_batch(BS) if li >= 1 else iter(())
                    weave(scan_ops(0), prev_b2)
                    for b in range(BS, BL):
                        conv(li, b)           # same layer, stream 1
                else:
                    # stream 0's DVE res weaves into stream 1's scan
                    weave(scan_ops(1), res_ops(0))
                    if li == len(DILATIONS) - 1:
                        for b in range(0, BS):
                            nc.sync.dma_start(o_d[b], X[:, b, PADX:SX])
                        for b in range(BS, BL):
                            for r in res_ops_batch(b):  # final: all-DVE
                                r()
                            nc.sync.dma_start(o_d[b], X[:, b, PADX:SX])
                    else:
                        for b in range(0, BS):
                            conv(li + 1, b)   # next layer, stream 0
                        res_act(BS + 1)       # batch 3 residual on ACT+PE

    nc.compile()
    return nc


def kernel(x, w, gamma, beta, mean, var, **_):
    from concourse.bass_utils import run_bass_kernel_spmd

    x = np.pad(np.asarray(x, np.float32), ((0, 0), (0, 0), (PADX, 0)))
    inv = (gamma / np.sqrt(var + EPS)).astype(np.float32)          # [4, C]
    # wt[ci, l, k, co] = 0.5 * w[l, co, ci, k] * inv[l, co]
    wt = (0.5 * w * inv[:, :, None, None]).astype(np.float32)      # [4, Co, Ci, K]
    wt = np.ascontiguousarray(wt.transpose(2, 0, 3, 1))            # [Ci, 4, K, Co]
    bias = (0.5 * (beta - mean * inv)).astype(np.float32).T        # [C, 4]
    bias = np.ascontiguousarray(bias)

    if "nc" not in _cache:
        _cache["nc"] = _build()
    nc = _cache["nc"]

    idm = np.eye(C, dtype=np.float32)
    in_maps = [
        {"x": np.ascontiguousarray(x[i * BL : (i + 1) * BL]), "wt": wt,
         "bias": bias, "idm": idm}
        for i in range(NCORES)
    ]
    res = run_bass_kernel_spmd(nc, in_maps, list(range(NCORES)))
    return np.concatenate([res.results[i]["out"] for i in range(NCORES)], axis=0)
